# revision 1
# baseline (speedup 1.0000x reference)
"""Trainium2 Bass kernel for CasimirSparseAttention.

Math (per batch b):
    S = (x_b @ x_b.T) / sqrt(D)                      # (T, T)
    probs = softmax(S, axis=-1)
    kept = probs >= 0.01  (vacuum = probs < 0.01)
    vac_sum = sum(probs * ~kept)
    casimir[t, o] = vac_sum[t] * rowsum_W[o]          # vac_in is const across D
    attended = (probs*kept) @ x_b / (sum(probs*kept) + 1e-9)
    out = attended + 0.01 * casimir

Working in unnormalized exp-space (E = exp(S/sqrt(D)), row sum = sa):
    kept mask:  E >= 0.01 * sa
    attended = (E*mask) @ x_b / (sum(E*mask) + 1e-9*sa)
    beta     = 0.01 * (1 - sum(E*mask)/sa);  out += beta * rowsum_W

Sharding: 8 cores = (batch b in 0..3) x (half of T). Each core computes
1024 query rows against all 2048 keys of its batch.

Per-core pipeline over 8 row-blocks of 128 queries:
    PE   : S-block via bf16 matmuls (xq^T stationary, x^T moving)
    ACT  : exp(scale*S) PSUM->SBUF with free-dim accumulate -> row sums
    DVE  : threshold mask (is_ge), masked E (bf16) + kept sums
    PE   : transpose masked-E chunks (128x128), then attended matmul
           in bf16 with hi/lo split of x for ~fp32 accuracy
    ACT  : scale attended rows by 1/(kept + 1e-9*sa)
    DVE  : add beta * rowsum_W rank-1 term

bf16 quantization of the masked weights cancels in attended because the
normalizer is computed from the same quantized values.
"""

import sys

sys.path.insert(0, "/opt/trn_rl_repo")

from contextlib import ExitStack

import numpy as np

from concourse import bacc, mybir, tile
from concourse.bass_utils import run_bass_kernel_spmd

F32 = mybir.dt.float32
BF16 = mybir.dt.bfloat16
OP = mybir.AluOpType
AFT = mybir.ActivationFunctionType

P = 128          # partitions / row-block size
T = 2048         # keys per batch
D = 1024         # model dim
QR = 1024        # query rows per core
NBLK = QR // P   # 8 row blocks per core
NC_CHUNKS = D // P    # 8 d-chunks
NT_CHUNKS = T // P    # 16 t-chunks
SCALE = float(1.0 / np.sqrt(np.float32(D)))   # 0.03125
THRESH = 0.01
EPS = 1e-9

_CACHE = {}


def _build():
    nc = bacc.Bacc("TRN2", target_bir_lowering=False, debug=False)

    FP8 = mybir.dt.float8e4
    # fp8 DoubleRow layout: [chunk, K=128, 2, N] pairs consecutive d-rows
    xt8 = nc.dram_tensor("xt8", [D // 256, P, 2, T], FP8, kind="ExternalInput")
    xq8 = nc.dram_tensor("xq8", [D // 256, P, 2, QR], FP8, kind="ExternalInput")
    xh = nc.dram_tensor("xh", [T, D], BF16, kind="ExternalInput")      # x_b hi
    xl = nc.dram_tensor("xl", [T, D], BF16, kind="ExternalInput")      # x_b lo
    wb = nc.dram_tensor("wb", [P, D], F32, kind="ExternalInput")       # rowsum_W bcast to 128 partitions
    ident = nc.dram_tensor("ident", [P, P], BF16, kind="ExternalInput")
    out = nc.dram_tensor("out", [QR, D], F32, kind="ExternalOutput")

    out_ap = out.ap()

    with tile.TileContext(nc) as tc, ExitStack() as ctx:
        # resident operands
        p_xt = ctx.enter_context(tc.tile_pool(name="xt", bufs=D // 256))
        p_xq = ctx.enter_context(tc.tile_pool(name="xq", bufs=D // 256))
        p_xn = ctx.enter_context(tc.tile_pool(name="xn", bufs=2 * NT_CHUNKS))
        p_cst = ctx.enter_context(tc.tile_pool(name="cst", bufs=2))
        # per-block working tiles
        p_exp = ctx.enter_context(tc.tile_pool(name="exp", bufs=5))
        p_msk = ctx.enter_context(tc.tile_pool(name="msk", bufs=5))
        p_pk = ctx.enter_context(tc.tile_pool(name="pk", bufs=6))
        p_pkt = ctx.enter_context(tc.tile_pool(name="pkt", bufs=3))
        p_out = ctx.enter_context(tc.tile_pool(name="o", bufs=2))
        p_wt = ctx.enter_context(tc.tile_pool(name="wt", bufs=2))
        p_sm = ctx.enter_context(tc.tile_pool(name="sm", bufs=40))
        # PSUM
        p_ps_s = ctx.enter_context(tc.tile_pool(name="ps_s", bufs=2, space="PSUM"))
        p_ps_a = ctx.enter_context(tc.tile_pool(name="ps_a", bufs=2, space="PSUM"))
        p_ps_t = ctx.enter_context(tc.tile_pool(name="ps_t", bufs=2, space="PSUM"))

        # S operands first (gate the first matmul), chunk pairs interleaved
        xq_sb, xt_sb = [], []
        for c in range(D // 256):
            tq = p_xq.tile([P, 2, QR], FP8, tag="xq", name="tq")
            nc.sync.dma_start(tq[:], xq8.ap()[c])
            xq_sb.append(tq)
            tt = p_xt.tile([P, 2, T], FP8, tag="xt", name="tt")
            nc.sync.dma_start(tt[:], xt8.ap()[c])
            xt_sb.append(tt)
        xh_sb, xl_sb = [], []
        for j in range(NT_CHUNKS):
            t_ = p_xn.tile([P, D], BF16, tag="xn")
            nc.sync.dma_start(t_[:], xh.ap()[j * P:(j + 1) * P, :])
            xh_sb.append(t_)
            t_ = p_xn.tile([P, D], BF16, tag="xn")
            nc.sync.dma_start(t_[:], xl.ap()[j * P:(j + 1) * P, :])
            xl_sb.append(t_)
        wb_sb = p_cst.tile([P, D], F32, tag="wb")
        nc.sync.dma_start(wb_sb[:], wb.ap())
        id_sb = p_cst.tile([P, P], BF16, tag="id")
        nc.sync.dma_start(id_sb[:], ident.ap())

        for i in range(NBLK):
            qcols = slice(i * P, (i + 1) * P)
            exp_halves, sum_parts, kept_f32_parts, keptq_parts, pk_halves = \
                [], [], [], [], []
            for half in range(2):
                sp = p_ps_s.tile([P, T // 2], F32, tag="s", name="sp")
                for k in range(2):
                    ncols = slice(half * (T // 2) + k * 512,
                                  half * (T // 2) + (k + 1) * 512)
                    for c in range(D // 256):
                        nc.tensor.matmul(
                            sp[:, k * 512:(k + 1) * 512],
                            lhsT=xq_sb[c][:, :, qcols],
                            rhs=xt_sb[c][:, :, ncols],
                            start=(c == 0), stop=(c == D // 256 - 1),
                            perf_mode=mybir.MatmulPerfMode.DoubleRow)
                ex = p_exp.tile([P, T // 2], F32, tag="ex")
                sa = p_sm.tile([P, 1], F32, tag="sm")
                nc.scalar.activation(ex[:], sp[:], AFT.Exp, scale=SCALE,
                                     accum_out=sa[:])
                exp_halves.append(ex)
                sum_parts.append(sa)

            sum_all = p_sm.tile([P, 1], F32, tag="sm")
            nc.gpsimd.tensor_tensor(sum_all[:], sum_parts[0][:],
                                    sum_parts[1][:], OP.add)
            thr = p_sm.tile([P, 1], F32, tag="sm")
            nc.gpsimd.tensor_scalar(out=thr[:], in0=sum_all[:],
                                    scalar1=THRESH, scalar2=None, op0=OP.mult)

            pkf_halves = []
            for half in range(2):
                mk = p_msk.tile([P, T // 2], F32, tag="mk")
                nc.vector.tensor_scalar(out=mk[:], in0=exp_halves[half][:],
                                        scalar1=thr[:], scalar2=None,
                                        op0=OP.is_ge)
                pkf = p_msk.tile([P, T // 2], F32, tag="pkf")
                nc.vector.tensor_tensor(pkf[:], exp_halves[half][:], mk[:],
                                        OP.mult)
                pk = p_pk.tile([P, T // 2], BF16, tag="pk")
                nc.scalar.copy(pk[:], pkf[:])
                pk_halves.append(pk)
                pkf_halves.append(pkf)

            att = [p_ps_a.tile([P, 512], F32, tag="a", name="att")
                   for _ in range(2)]
            # transpose groups into PSUM, one wide copy out per group;
            # first group is a singleton so attended starts sooner
            groups = [[0], [1, 2, 3]] + [
                list(range(g, g + 4)) for g in range(4, NT_CHUNKS, 4)]
            for grp in groups:
                tp = p_ps_t.tile([P, len(grp) * P], BF16, tag="t", name="tp")
                for jj, j in enumerate(grp):
                    src = pk_halves[j // 8][:, (j % 8) * P:(j % 8 + 1) * P]
                    nc.tensor.transpose(tp[:, jj * P:(jj + 1) * P], src,
                                        id_sb[:])
                pkt = p_pkt.tile([P, len(grp) * P], BF16, tag="pkt",
                                 name="pkt")
                nc.scalar.copy(pkt[:], tp[:])
                for jj, j in enumerate(grp):
                    lhs = pkt[:, jj * P:(jj + 1) * P]
                    for k in range(2):
                        dcols = slice(k * 512, (k + 1) * 512)
                        nc.tensor.matmul(att[k][:], lhsT=lhs,
                                         rhs=xh_sb[j][:, dcols],
                                         start=(j == 0), stop=False)
                        nc.tensor.matmul(att[k][:], lhsT=lhs,
                                         rhs=xl_sb[j][:, dcols],
                                         start=False,
                                         stop=(j == NT_CHUNKS - 1))

            # deferred kept-sum reduces (off the critical path to attended)
            for half in range(2):
                kf = p_sm.tile([P, 1], F32, tag="sm", name="kf")
                nc.vector.tensor_reduce(kf[:], pkf_halves[half][:],
                                        mybir.AxisListType.X, OP.add)
                kq = p_sm.tile([P, 1], F32, tag="sm", name="kq")
                nc.vector.tensor_reduce(kq[:], pk_halves[half][:],
                                        mybir.AxisListType.X, OP.add)
                kept_f32_parts.append(kf)
                keptq_parts.append(kq)
            # kept sums: f32 (matches reference) for beta, quantized for alpha
            kept_f = p_sm.tile([P, 1], F32, tag="sm")
            nc.gpsimd.tensor_tensor(kept_f[:], kept_f32_parts[0][:],
                                    kept_f32_parts[1][:], OP.add)
            kept_q = p_sm.tile([P, 1], F32, tag="sm")
            nc.gpsimd.tensor_tensor(kept_q[:], keptq_parts[0][:],
                                    keptq_parts[1][:], OP.add)

            # alpha = 1 / (kept_q + eps * sum_all)
            den = p_sm.tile([P, 1], F32, tag="sm")
            nc.gpsimd.tensor_scalar(out=den[:], in0=sum_all[:], scalar1=EPS,
                                    scalar2=None, op0=OP.mult)
            nc.gpsimd.tensor_tensor(den[:], den[:], kept_q[:], OP.add)
            alpha = p_sm.tile([P, 1], F32, tag="sm")
            nc.vector.reciprocal(alpha[:], den[:])
            # beta = 0.01 * (1 - kept_f / sum_all)
            rsum = p_sm.tile([P, 1], F32, tag="sm")
            nc.vector.reciprocal(rsum[:], sum_all[:])
            beta = p_sm.tile([P, 1], F32, tag="sm")
            nc.gpsimd.tensor_tensor(beta[:], kept_f[:], rsum[:], OP.mult)
            nc.gpsimd.tensor_scalar(out=beta[:], in0=beta[:], scalar1=-THRESH,
                                    scalar2=THRESH, op0=OP.mult, op1=OP.add)

            o_sb = p_out.tile([P, D], F32, tag="o")
            for k in range(2):
                dcols = slice(k * 512, (k + 1) * 512)
                nc.scalar.mul(o_sb[:, dcols], att[k][:], alpha[:])
            wt = p_wt.tile([P, D], F32, tag="wt")
            nc.vector.tensor_scalar(out=wt[:], in0=wb_sb[:], scalar1=beta[:],
                                    scalar2=None, op0=OP.mult)
            nc.vector.tensor_tensor(o_sb[:], o_sb[:], wt[:], OP.add)
            nc.sync.dma_start(out_ap[i * P:(i + 1) * P, :], o_sb[:])

    nc.compile()
    return nc


def get_nc():
    if "nc" not in _CACHE:
        _CACHE["nc"] = _build()
    return _CACHE["nc"]


def make_in_maps(x, W):
    import ml_dtypes
    bf = ml_dtypes.bfloat16
    f8 = ml_dtypes.float8_e4m3
    x = np.asarray(x, dtype=np.float32)
    W = np.asarray(W, dtype=np.float32)
    wrow = W.sum(axis=1, dtype=np.float32)                      # (D,)
    wb = np.ascontiguousarray(np.broadcast_to(wrow, (P, D))).astype(np.float32)
    ident = np.eye(P, dtype=bf)
    in_maps = []
    for core in range(8):
        b, h = core // 2, core % 2
        xb = x[b]                                               # (T, D)
        xt_f8 = np.ascontiguousarray(xb.T).astype(f8)           # (D, T)
        xt8 = xt_f8.reshape(D // 256, P, 2, T)
        xq8 = np.ascontiguousarray(xt8[:, :, :, h * QR:(h + 1) * QR])
        xh_bf = xb.astype(bf)
        xl_bf = (xb - xh_bf.astype(np.float32)).astype(bf)
        in_maps.append({"xt8": xt8, "xq8": xq8, "xh": xh_bf,
                        "xl": xl_bf, "wb": wb, "ident": ident})
    return in_maps


def kernel(x, W):
    nc = get_nc()
    in_maps = make_in_maps(x, W)
    res = run_bass_kernel_spmd(nc, in_maps, list(range(8)))
    out = np.empty((4, T, D), dtype=np.float32)
    for core in range(8):
        b, h = core // 2, core % 2
        out[b, h * QR:(h + 1) * QR, :] = res.results[core]["out"]
    return out



# revision 33
# speedup vs baseline: 2.4530x; 2.4530x over previous
"""Trainium2 Bass kernel for CasimirSparseAttention.

Math (per batch b):
    S = (x_b @ x_b.T) / sqrt(D)                      # (T, T)
    probs = softmax(S, axis=-1)
    kept = probs >= 0.01  (vacuum = probs < 0.01)
    vac_sum = sum(probs * ~kept)
    casimir[t, o] = vac_sum[t] * rowsum_W[o]          # vac_in is const across D
    attended = (probs*kept) @ x_b / (sum(probs*kept) + 1e-9)
    out = attended + 0.01 * casimir

Working in unnormalized exp-space (E = exp(S/sqrt(D)), row sum = sa):
    kept mask:  E >= 0.01 * sa
    w = (E * mask) / sa            # masked probs, cast to fp8 e4m3
    kq = sum(w8)                   # fp8-consistent normalizer (matmul ones col)
    attended = w8 @ (xh8 + xl8) / (kq + 1e-9)   # hi/lo fp8 split of x
    out = attended + 0.01*(1 - kq) * rowsum_W

fp8 weight quantization cancels in attended because the normalizer kq is
the sum of the SAME fp8 values the matmul contracts (ones-column matmul).
x is split hi/lo in fp8 e4m3 (xh8 = fp8(x), xl8 = fp8(x - xh8)), giving
~bf16-level value accuracy at fp8 DoubleRow matmul throughput.

Sharding: 8 cores = (batch b in 0..3) x (half of T). Keys are rotated
per-core so this core's 1024 query rows are always key columns 0:1024 —
the S lhsT is then a plain slice of the resident key tensor (no separate
query DMA). The attended values use the same rotated key order; output
rows are queries, so the rotation needs no undo.

Per-core, software-pipelined 2 blocks deep over 8 row-blocks of 128 rows.
Emission order per step interleaves PE stages of block i-2 with the S
matmuls of block i so the DVE->PE handoffs (pk8, pkt) are latency-hidden:
    D(i-2): PE   16 fp8 128x128 transposes of pk8 into one PSUM bank
    A(i):   PE   S-block via fp8 DoubleRow matmuls
    E(i-2): DVE  single bitcast-u16 PSUM->SBUF copy of the transposes
    F(i-2): PE   kq ones-matmul + 32 fp8 DoubleRow att matmuls (hi+lo)
    B(i):   ACT  exp(scale*S) -> bf16 + row-sum accum; Pool thr; DVE 1/sa
    C(i):   DVE  mk = (E>=thr)*rsum (4x tensor_scalar), pk8 = E*mk -> fp8
    G(i-2): Pool/DVE alpha = 1/(kq+1e-9), beta = 0.01*(1-kq)
    H(i-2): ACT  o = att*alpha -> bf16; Pool wt = wb*beta, o2 = o+wt; DMA out
"""

import sys

sys.path.insert(0, "/opt/trn_rl_repo")

from contextlib import ExitStack

import numpy as np

from concourse import bacc, mybir, tile
from concourse.bass_utils import run_bass_kernel_spmd

F32 = mybir.dt.float32
BF16 = mybir.dt.bfloat16
FP8 = mybir.dt.float8e4
U16 = mybir.dt.uint16
F16 = mybir.dt.float16
OP = mybir.AluOpType
AFT = mybir.ActivationFunctionType
DR = mybir.MatmulPerfMode.DoubleRow

P = 128          # partitions / row-block size
T = 2048         # keys per batch
D = 1024         # model dim
QR = 1024        # query rows per core
NBLK = QR // P   # 8 row blocks per core
NKC = T // 256   # 8 contraction chunks (256 wide) for attended
SCALE = float(1.0 / np.sqrt(np.float32(D)))   # 0.03125
THRESH = 0.01
EPS = 1e-9

_CACHE = {}


def _build():
    nc = bacc.Bacc("TRN2", target_bir_lowering=False, debug=False)

    # fp8 DoubleRow layout for S, split in column halves for earlier S start:
    # [chunk, K=128, 2, T/2], pairs = rows 2p, 2p+1 of x^T (rotated key order)
    xt0 = nc.dram_tensor("xt0", [D // 256, P, 2, T // 2], FP8,
                         kind="ExternalInput")
    xt1 = nc.dram_tensor("xt1", [D // 256, P, 2, T // 2], FP8,
                         kind="ExternalInput")
    # fp8 DoubleRow value layout for attended: [chunk, K=128, 2, D],
    # element [c, p, j, :] = x[c*256 + j*128 + p, :] in rotated key order
    # (hi = fp8(x), lo = fp8(x - hi))
    xh8 = nc.dram_tensor("xh8", [NKC, P, 2, D], FP8, kind="ExternalInput")
    xl8 = nc.dram_tensor("xl8", [NKC, P, 2, D], FP8, kind="ExternalInput")
    wb = nc.dram_tensor("wb", [P, D], BF16, kind="ExternalInput")  # rowsum_W
    idb = nc.dram_tensor("idb", [P, P], BF16, kind="ExternalInput")
    on8 = nc.dram_tensor("on8", [P, 2, 4], FP8, kind="ExternalInput")
    out = nc.dram_tensor("out", [QR, D], BF16, kind="ExternalOutput")

    out_ap = out.ap()

    with tile.TileContext(nc) as tc, ExitStack() as ctx:
        # resident operands
        p_xt = ctx.enter_context(tc.tile_pool(name="xt", bufs=2 * (D // 256)))
        p_xv = ctx.enter_context(tc.tile_pool(name="xv", bufs=2 * NKC))
        p_cst = ctx.enter_context(tc.tile_pool(name="cst", bufs=3))
        # per-block working tiles
        p_ex = ctx.enter_context(tc.tile_pool(name="ex", bufs=4))
        p_mk = ctx.enter_context(tc.tile_pool(name="mk", bufs=4))
        p_pk = ctx.enter_context(tc.tile_pool(name="pk", bufs=6))
        p_pkt = ctx.enter_context(tc.tile_pool(name="pkt", bufs=2))
        p_o = ctx.enter_context(tc.tile_pool(name="o", bufs=2))
        p_o2 = ctx.enter_context(tc.tile_pool(name="o2", bufs=2))
        p_wt = ctx.enter_context(tc.tile_pool(name="wt", bufs=2))
        p_sm = ctx.enter_context(tc.tile_pool(name="sm", bufs=56))
        # PSUM: 2x(2 banks) S halves + 1 bank transposes + 2 banks att + kq
        p_ps_s = ctx.enter_context(tc.tile_pool(name="ps_s", bufs=2,
                                                space="PSUM"))
        p_ps_tp = ctx.enter_context(tc.tile_pool(name="ps_tp", bufs=1,
                                                 space="PSUM"))
        p_ps_a = ctx.enter_context(tc.tile_pool(name="ps_a", bufs=1,
                                                space="PSUM"))
        p_ps_k = ctx.enter_context(tc.tile_pool(name="ps_k", bufs=1,
                                                space="PSUM"))

        # constants + S operands first (gate the first matmul); all DMAs share
        # one 360GB/s pipe, so order = priority.
        id_sb = p_cst.tile([P, P], BF16, tag="id")
        nc.sync.dma_start(id_sb[:], idb.ap())
        on_sb = p_cst.tile([P, 2, 4], FP8, tag="on")
        nc.sync.dma_start(on_sb[:], on8.ap())
        xt_sb = [[], []]          # [half][chunk]
        for half, dram in ((0, xt0), (1, xt1)):
            for c in range(D // 256):
                t_ = p_xt.tile([P, 2, T // 2], FP8, tag="xt", name="t_xt")
                nc.sync.dma_start(t_[:], dram.ap()[c])
                xt_sb[half].append(t_)
        wb_sb = p_cst.tile([P, D], BF16, tag="wb")
        nc.sync.dma_start(wb_sb[:], wb.ap())
        # value loads ordered to match F-stage group consumption:
        # hi c0-3, lo c0-3, hi c4-7, lo c4-7
        xh_sb, xl_sb = [None] * NKC, [None] * NKC
        for grp in range(2):
            cs = range(grp * (NKC // 2), (grp + 1) * (NKC // 2))
            for sb, dram, nm in ((xh_sb, xh8, "t_xh"), (xl_sb, xl8, "t_xl")):
                for c in cs:
                    t_ = p_xv.tile([P, 2, D], FP8, tag="xv", name=nm)
                    nc.sync.dma_start(t_[:], dram.ap()[c])
                    sb[c] = t_

        # per-block state carried across pipeline stages
        pk8_blk = [None] * NBLK
        tp_blk = [None] * NBLK

        def emit_d(i, grp):
            # 8 bf16 transposes of t-segments grp*8..grp*8+7 into one
            # 1-bank PSUM tile: plane s holds keys t = grp*1024 + s*128 + p
            # for 128 contiguous q columns.
            tp = p_ps_tp.tile([P, NKC, P], BF16, tag="tp", name="tp")
            for s in range(8):
                seg = grp * 8 + s
                src = pk8_blk[i][seg // 8][:, (seg % 8) * P:(seg % 8 + 1) * P]
                nc.tensor.transpose(tp[:, s, :], src, id_sb[:])
            tp_blk[i] = tp

        def emit_a(i, halves_sel, state):
            # S by column halves; `halves_sel` selects which to emit so
            # callers can interleave other PE work between them.
            qcols = slice(i * P, (i + 1) * P)
            if state is None:
                state = ([None, None], [])
            ex_halves, sa_parts = state
            for half in halves_sel:
                sp = p_ps_s.tile([P, T // 2], F32, tag="s", name="sp")
                for k in range(2):
                    for c in range(D // 256):
                        nc.tensor.matmul(
                            sp[:, k * 512:(k + 1) * 512],
                            lhsT=xt_sb[0][c][:, :, qcols],
                            rhs=xt_sb[half][c][:, :, k * 512:(k + 1) * 512],
                            start=(c == 0), stop=(c == D // 256 - 1),
                            perf_mode=DR)
                ex = p_ex.tile([P, T // 2], BF16, tag="ex", name="ex")
                sa = p_sm.tile([P, 1], F32, tag="sm")
                nc.scalar.activation(ex[:], sp[:], AFT.Exp, scale=SCALE,
                                     accum_out=sa[:])
                ex_halves[half] = ex
                sa_parts.append(sa)
            return state

        def emit_bc(i, ex_halves, sa_parts):
            sa = p_sm.tile([P, 1], F32, tag="sm")
            nc.gpsimd.tensor_tensor(sa[:], sa_parts[0][:], sa_parts[1][:],
                                    OP.add)
            thr = p_sm.tile([P, 1], F32, tag="sm")
            nc.gpsimd.tensor_scalar(out=thr[:], in0=sa[:], scalar1=THRESH,
                                    scalar2=None, op0=OP.mult)
            rsum = p_sm.tile([P, 1], F32, tag="sm")
            nc.vector.reciprocal(rsum[:], sa[:])
            halves = []
            for half in range(2):
                mk = p_mk.tile([P, T // 2], BF16, tag="mk")
                nc.vector.tensor_scalar(out=mk[:], in0=ex_halves[half][:],
                                        scalar1=thr[:], scalar2=rsum[:],
                                        op0=OP.is_ge, op1=OP.mult)
                pkb = p_pk.tile([P, T // 2], BF16, tag="pk")
                nc.vector.tensor_tensor(pkb[:], ex_halves[half][:], mk[:],
                                        OP.mult)
                halves.append(pkb)
            pk8_blk[i] = halves

        def emit_e(i, grp, state):
            # copy+cast bf16 -> fp8: planes [p, s, q] with contiguous q,
            # giving ISA-valid dual-fp8 DoubleRow weights [p, 2, q].
            # Split DVE/ACT so the copy latency hides under the S matmuls.
            if state is None:
                kqp = p_ps_k.tile([P, 4], F32, tag="kq", name="kqp")
                att = p_ps_a.tile([P, D], F32, tag="a", name="att")
                state = ([None, None], kqp, att)
            pkt = p_pkt.tile([P, NKC, P], FP8, tag="pkt", name="pkt")
            tp = tp_blk[i]
            nc.vector.tensor_copy(pkt[:, 0:4], tp[:, 0:4])
            nc.scalar.copy(pkt[:, 4:8], tp[:, 4:8])
            state[0][grp] = pkt
            return state

        def emit_f(i, grp, state):
            # kq + att matmuls for this group's 4 contraction chunks.
            pkts, kqp, att = state
            pkt = pkts[grp]
            for cl in range(NKC // 2):
                c = grp * (NKC // 2) + cl
                lhs = pkt[:, 2 * cl:2 * cl + 2, :]
                nc.tensor.matmul(kqp[:], lhsT=lhs,
                                 rhs=on_sb[:], start=(c == 0),
                                 stop=(c == NKC - 1), perf_mode=DR)
                for src in (xh_sb, xl_sb):
                    for k in range(2):
                        nc.tensor.matmul(
                            att[:, k * 512:(k + 1) * 512],
                            lhsT=lhs,
                            rhs=src[c][:, :, k * 512:(k + 1) * 512],
                            start=(c == 0 and src is xh_sb),
                            stop=(c == NKC - 1 and src is xl_sb),
                            perf_mode=DR)
            return state

        def emit_gh(i, kqp, att):
            den = p_sm.tile([P, 1], F32, tag="sm")
            nc.vector.tensor_scalar(out=den[:], in0=kqp[:, 0:1], scalar1=EPS,
                                    scalar2=None, op0=OP.add)
            alpha = p_sm.tile([P, 1], F32, tag="sm")
            nc.vector.reciprocal(alpha[:], den[:])
            beta = p_sm.tile([P, 1], F32, tag="sm")
            nc.vector.tensor_scalar(out=beta[:], in0=kqp[:, 0:1],
                                    scalar1=-THRESH, scalar2=THRESH,
                                    op0=OP.mult, op1=OP.add)
            o = p_o.tile([P, D], BF16, tag="o")
            nc.scalar.mul(o[:], att[:], alpha[:])
            wt = p_wt.tile([P, D], BF16, tag="wt")
            nc.gpsimd.tensor_scalar(out=wt[:], in0=wb_sb[:], scalar1=beta[:],
                                    scalar2=None, op0=OP.mult)
            o2 = p_o2.tile([P, D], BF16, tag="o2")
            nc.gpsimd.tensor_tensor(o2[:], o[:], wt[:], OP.add)
            nc.sync.dma_start(out_ap[i * P:(i + 1) * P, :], o2[:])

        abc_state = [None] * NBLK
        for step in range(NBLK + 3):
            j = step - 3
            if j >= 0:
                emit_d(j, 0)
            if step < NBLK:
                abc_state[step] = emit_a(step, (0,), None)
            if j >= 0:
                ef = emit_e(j, 0, None)
                emit_f(j, 0, ef)
                emit_d(j, 1)
            if step < NBLK:
                emit_a(step, (1,), abc_state[step])
            if j >= 0:
                emit_e(j, 1, ef)
                emit_f(j, 1, ef)
            if step < NBLK:
                emit_bc(step, *abc_state[step])
            if j >= 0:
                emit_gh(j, ef[1], ef[2])

    nc.compile()
    return nc


def get_nc():
    if "nc" not in _CACHE:
        _CACHE["nc"] = _build()
    return _CACHE["nc"]


def make_in_maps(x, W):
    import ml_dtypes
    bf = ml_dtypes.bfloat16
    f8 = ml_dtypes.float8_e4m3
    x = np.asarray(x, dtype=np.float32)
    W = np.asarray(W, dtype=np.float32)
    wrow = W.sum(axis=1, dtype=np.float32)                      # (D,)
    wb = np.ascontiguousarray(np.broadcast_to(wrow, (P, D))).astype(bf)
    idb = np.eye(P, dtype=bf)
    on8 = np.ones((P, 2, 4), dtype=f8)
    in_maps = []
    for core in range(8):
        b, h = core // 2, core % 2
        # rotate keys so this core's queries are key columns 0:QR
        xb = np.roll(x[b], -h * QR, axis=0)                     # (T, D)
        xt8 = np.ascontiguousarray(xb.T).astype(f8).reshape(D // 256, P, 2, T)
        xt0 = np.ascontiguousarray(xt8[:, :, :, :T // 2])
        xt1 = np.ascontiguousarray(xt8[:, :, :, T // 2:])
        xh8_f = xb.astype(f8)                                   # (T, D)
        xl8_f = (xb - xh8_f.astype(np.float32)).astype(f8)
        # DoubleRow pairing: lhsT partition p, slot j of chunk c holds key
        # t = c*256 + j*128 + p
        xh8 = np.ascontiguousarray(
            xh8_f.reshape(NKC, 2, P, D).transpose(0, 2, 1, 3))
        xl8 = np.ascontiguousarray(
            xl8_f.reshape(NKC, 2, P, D).transpose(0, 2, 1, 3))
        in_maps.append({"xt0": xt0, "xt1": xt1, "xh8": xh8, "xl8": xl8,
                        "wb": wb, "idb": idb, "on8": on8})
    return in_maps


def kernel(x, W):
    nc = get_nc()
    in_maps = make_in_maps(x, W)
    res = run_bass_kernel_spmd(nc, in_maps, list(range(8)))
    out = np.empty((4, T, D), dtype=np.float32)
    for core in range(8):
        b, h = core // 2, core % 2
        out[b, h * QR:(h + 1) * QR, :] = \
            np.asarray(res.results[core]["out"]).astype(np.float32)
    return out


# revision 37
# speedup vs baseline: 2.6275x; 1.0712x over previous
"""Trainium2 Bass kernel for CasimirSparseAttention.

Math (per batch b):
    S = (x_b @ x_b.T) / sqrt(D)                      # (T, T)
    probs = softmax(S, axis=-1)
    kept = probs >= 0.01  (vacuum = probs < 0.01)
    vac_sum = sum(probs * ~kept)
    casimir[t, o] = vac_sum[t] * rowsum_W[o]          # vac_in is const across D
    attended = (probs*kept) @ x_b / (sum(probs*kept) + 1e-9)
    out = attended + 0.01 * casimir

Working in unnormalized exp-space (E = exp(S/sqrt(D)), row sum = sa):
    kept mask:  E >= 0.01 * sa
    w = (E * mask) / sa            # masked probs, cast to fp8 e4m3
    kq = sum(w8)                   # fp8-consistent normalizer (matmul ones col)
    attended = w8 @ (xh8 + xl8) / (kq + 1e-9)   # hi/lo fp8 split of x
    out = attended + 0.01*(1 - kq) * rowsum_W

fp8 weight quantization cancels in attended because the normalizer kq is
the sum of the SAME fp8 values the matmul contracts (ones-column matmul).
x is split hi/lo in fp8 e4m3 (xh8 = fp8(x), xl8 = fp8(x - xh8)), giving
~bf16-level value accuracy at fp8 DoubleRow matmul throughput.

Sharding: 8 cores = (batch b in 0..3) x (half of T). Keys are rotated
per-core so this core's 1024 query rows are always key columns 0:1024 —
the S lhsT is then a plain slice of the resident key tensor (no separate
query DMA). The attended values use the same rotated key order; output
rows are queries, so the rotation needs no undo.

Per-core, software-pipelined 3 blocks deep over 8 row-blocks of 128 rows.
Emission order per step interleaves PE stages of block i-3 with the S
matmuls of block i so the DVE/ACT->PE handoffs (pkb, pkt) and the input
DMA fill are latency-hidden. Stages (j = i-3), each in two groups of 4
contraction chunks:
    D(j):   PE   8 bf16 128x128 transposes of pkb into one PSUM bank
    A(i):   PE   S-column-half via fp8 DoubleRow matmuls; ACT exp->bf16
                 + row-sum accum (B: Pool thr; DVE 1/sa)
    E(j):   DVE+ACT copy transposes PSUM->SBUF casting bf16->fp8
                 (plane-contiguous => ISA-valid dual-fp8 ldweights)
    F(j):   PE   kq ones-matmul + fp8 DoubleRow att matmuls (hi+lo)
    C(i):   DVE  mk = (E>=thr)*rsum (4x tensor_scalar), pkb = E*mk (bf16)
    G(j):   DVE  alpha = 1/(kq+1e-9), beta = 0.01*(1-kq)
    H(j):   ACT  o = att*alpha -> bf16; Pool wt = wb*beta, o2 = o+wt;
                 DMA out

The fp8 PE transpose writes element-step-2 output and dual-fp8 ldweights
require contiguous columns, so weights transpose in bf16 and the fp8 cast
rides the PSUM->SBUF copy.
"""

import sys

sys.path.insert(0, "/opt/trn_rl_repo")

from contextlib import ExitStack

import numpy as np

from concourse import bacc, mybir, tile
from concourse.bass_utils import run_bass_kernel_spmd

F32 = mybir.dt.float32
BF16 = mybir.dt.bfloat16
FP8 = mybir.dt.float8e4
U16 = mybir.dt.uint16
F16 = mybir.dt.float16
OP = mybir.AluOpType
AFT = mybir.ActivationFunctionType
DR = mybir.MatmulPerfMode.DoubleRow

P = 128          # partitions / row-block size
T = 2048         # keys per batch
D = 1024         # model dim
QR = 1024        # query rows per core
NBLK = QR // P   # 8 row blocks per core
NKC = T // 256   # 8 contraction chunks (256 wide) for attended
SCALE = float(1.0 / np.sqrt(np.float32(D)))   # 0.03125
THRESH = 0.01
EPS = 1e-9

_CACHE = {}


def _build():
    nc = bacc.Bacc("TRN2", target_bir_lowering=False, debug=False)

    # fp8 DoubleRow layout for S, split in column halves for earlier S start:
    # [chunk, K=128, 2, T/2], pairs = rows 2p, 2p+1 of x^T (rotated key order)
    xt0 = nc.dram_tensor("xt0", [D // 256, P, 2, T // 2], FP8,
                         kind="ExternalInput")
    xt1 = nc.dram_tensor("xt1", [D // 256, P, 2, T // 2], FP8,
                         kind="ExternalInput")
    # fp8 DoubleRow value layout for attended: [chunk, K=128, 2, D],
    # element [c, p, j, :] = x[c*256 + j*128 + p, :] in rotated key order
    # (hi = fp8(x), lo = fp8(x - hi))
    xh8 = nc.dram_tensor("xh8", [NKC, P, 2, D], FP8, kind="ExternalInput")
    # bf16 residuals (x - fp8(x)) for this core's own 1024 keys, one
    # [128, D] tile per q-block's diagonal segment
    rdg = nc.dram_tensor("rdg", [NBLK, P, D], BF16, kind="ExternalInput")
    wb = nc.dram_tensor("wb", [P, D], BF16, kind="ExternalInput")  # rowsum_W
    idb = nc.dram_tensor("idb", [P, P], BF16, kind="ExternalInput")
    on8 = nc.dram_tensor("on8", [P, 2, 4], FP8, kind="ExternalInput")
    out = nc.dram_tensor("out", [QR, D], BF16, kind="ExternalOutput")

    out_ap = out.ap()

    with tile.TileContext(nc) as tc, ExitStack() as ctx:
        # resident operands
        p_xt = ctx.enter_context(tc.tile_pool(name="xt", bufs=2 * (D // 256)))
        p_xv = ctx.enter_context(tc.tile_pool(name="xv", bufs=NKC))
        p_rd = ctx.enter_context(tc.tile_pool(name="rd", bufs=NBLK))
        p_pkd = ctx.enter_context(tc.tile_pool(name="pkd", bufs=2))
        p_cst = ctx.enter_context(tc.tile_pool(name="cst", bufs=3))
        # per-block working tiles
        p_ex = ctx.enter_context(tc.tile_pool(name="ex", bufs=4))
        p_mk = ctx.enter_context(tc.tile_pool(name="mk", bufs=4))
        p_pk = ctx.enter_context(tc.tile_pool(name="pk", bufs=6))
        p_pkt = ctx.enter_context(tc.tile_pool(name="pkt", bufs=2))
        p_o = ctx.enter_context(tc.tile_pool(name="o", bufs=2))
        p_o2 = ctx.enter_context(tc.tile_pool(name="o2", bufs=2))
        p_wt = ctx.enter_context(tc.tile_pool(name="wt", bufs=2))
        p_sm = ctx.enter_context(tc.tile_pool(name="sm", bufs=56))
        # PSUM: 2x(2 banks) S halves + 1 bank transposes + 2 banks att + kq
        p_ps_s = ctx.enter_context(tc.tile_pool(name="ps_s", bufs=2,
                                                space="PSUM"))
        p_ps_tp = ctx.enter_context(tc.tile_pool(name="ps_tp", bufs=1,
                                                 space="PSUM"))
        p_ps_a = ctx.enter_context(tc.tile_pool(name="ps_a", bufs=1,
                                                space="PSUM"))
        p_ps_k = ctx.enter_context(tc.tile_pool(name="ps_k", bufs=1,
                                                space="PSUM"))

        # constants + S operands first (gate the first matmul); all DMAs share
        # one 360GB/s pipe, so order = priority.
        id_sb = p_cst.tile([P, P], BF16, tag="id")
        nc.sync.dma_start(id_sb[:], idb.ap())
        on_sb = p_cst.tile([P, 2, 4], FP8, tag="on")
        nc.sync.dma_start(on_sb[:], on8.ap())
        xt_sb = [[], []]          # [half][chunk]
        for half, dram in ((0, xt0), (1, xt1)):
            for c in range(D // 256):
                t_ = p_xt.tile([P, 2, T // 2], FP8, tag="xt", name="t_xt")
                nc.sync.dma_start(t_[:], dram.ap()[c])
                xt_sb[half].append(t_)
        wb_sb = p_cst.tile([P, D], BF16, tag="wb")
        nc.sync.dma_start(wb_sb[:], wb.ap())
        # value loads: hi chunks, then per-block diag residuals
        xh_sb, rd_sb = [], []
        for c in range(NKC):
            t_ = p_xv.tile([P, 2, D], FP8, tag="xv", name="t_xh")
            nc.sync.dma_start(t_[:], xh8.ap()[c])
            xh_sb.append(t_)
        for i in range(NBLK):
            t_ = p_rd.tile([P, D], BF16, tag="rd", name="t_rd")
            nc.sync.dma_start(t_[:], rdg.ap()[i])
            rd_sb.append(t_)

        # per-block state carried across pipeline stages
        pk8_blk = [None] * NBLK
        tp_blk = [None] * NBLK

        def emit_d(i, grp):
            # 8 bf16 transposes of t-segments grp*8..grp*8+7 into one
            # 1-bank PSUM tile: plane s holds keys t = grp*1024 + s*128 + p
            # for 128 contiguous q columns.
            tp = p_ps_tp.tile([P, NKC, P], BF16, tag="tp", name="tp")
            for s in range(8):
                seg = grp * 8 + s
                src = pk8_blk[i][seg // 8][:, (seg % 8) * P:(seg % 8 + 1) * P]
                nc.tensor.transpose(tp[:, s, :], src, id_sb[:])
            tp_blk[i] = tp

        def emit_a(i, halves_sel, state):
            # S by column halves; `halves_sel` selects which to emit so
            # callers can interleave other PE work between them.
            qcols = slice(i * P, (i + 1) * P)
            if state is None:
                state = ([None, None], [])
            ex_halves, sa_parts = state
            for half in halves_sel:
                sp = p_ps_s.tile([P, T // 2], F32, tag="s", name="sp")
                for k in range(2):
                    for c in range(D // 256):
                        nc.tensor.matmul(
                            sp[:, k * 512:(k + 1) * 512],
                            lhsT=xt_sb[0][c][:, :, qcols],
                            rhs=xt_sb[half][c][:, :, k * 512:(k + 1) * 512],
                            start=(c == 0), stop=(c == D // 256 - 1),
                            perf_mode=DR)
                ex = p_ex.tile([P, T // 2], BF16, tag="ex", name="ex")
                sa = p_sm.tile([P, 1], F32, tag="sm")
                nc.scalar.activation(ex[:], sp[:], AFT.Exp, scale=SCALE,
                                     accum_out=sa[:])
                ex_halves[half] = ex
                sa_parts.append(sa)
            return state

        def emit_bc(i, ex_halves, sa_parts):
            sa = p_sm.tile([P, 1], F32, tag="sm")
            nc.gpsimd.tensor_tensor(sa[:], sa_parts[0][:], sa_parts[1][:],
                                    OP.add)
            thr = p_sm.tile([P, 1], F32, tag="sm")
            nc.gpsimd.tensor_scalar(out=thr[:], in0=sa[:], scalar1=THRESH,
                                    scalar2=None, op0=OP.mult)
            rsum = p_sm.tile([P, 1], F32, tag="sm")
            nc.vector.reciprocal(rsum[:], sa[:])
            halves = []
            for half in range(2):
                mk = p_mk.tile([P, T // 2], BF16, tag="mk")
                nc.vector.tensor_scalar(out=mk[:], in0=ex_halves[half][:],
                                        scalar1=thr[:], scalar2=rsum[:],
                                        op0=OP.is_ge, op1=OP.mult)
                pkb = p_pk.tile([P, T // 2], BF16, tag="pk")
                nc.vector.tensor_tensor(pkb[:], ex_halves[half][:], mk[:],
                                        OP.mult)
                halves.append(pkb)
            pk8_blk[i] = halves

        def emit_e(i, grp, state):
            # copy+cast bf16 -> fp8: planes [p, s, q] with contiguous q,
            # giving ISA-valid dual-fp8 DoubleRow weights [p, 2, q].
            # Split DVE/ACT so the copy latency hides under the S matmuls.
            if state is None:
                kqp = p_ps_k.tile([P, 4], F32, tag="kq", name="kqp")
                att = p_ps_a.tile([P, D], F32, tag="a", name="att")
                state = ([None, None], kqp, att, None)
            pkt = p_pkt.tile([P, NKC, P], FP8, tag="pkt", name="pkt")
            tp = tp_blk[i]
            nc.vector.tensor_copy(pkt[:, 0:4], tp[:, 0:4])
            nc.scalar.copy(pkt[:, 4:8], tp[:, 4:8])
            if grp == 0:
                # diag segment of block i is plane i of group 0: keep the
                # bf16 weights for the residual correction matmul
                pkd = p_pkd.tile([P, P], BF16, tag="pkd", name="pkd")
                nc.vector.tensor_copy(pkd[:], tp[:, i, :])
                state = (state[0], state[1], state[2], pkd)
            state[0][grp] = pkt
            return state

        def emit_f(i, grp, state):
            # kq + att matmuls for this group's 4 contraction chunks,
            # plus the bf16 diagonal-residual correction after group 0.
            pkts, kqp, att, pkd = state
            pkt = pkts[grp]
            for cl in range(NKC // 2):
                c = grp * (NKC // 2) + cl
                lhs = pkt[:, 2 * cl:2 * cl + 2, :]
                nc.tensor.matmul(kqp[:], lhsT=lhs,
                                 rhs=on_sb[:], start=(c == 0),
                                 stop=(c == NKC - 1), perf_mode=DR)
                for k in range(2):
                    nc.tensor.matmul(
                        att[:, k * 512:(k + 1) * 512],
                        lhsT=lhs,
                        rhs=xh_sb[c][:, :, k * 512:(k + 1) * 512],
                        start=(c == 0), stop=(c == NKC - 1),
                        perf_mode=DR)
            if grp == 0:
                for k in range(2):
                    nc.tensor.matmul(
                        att[:, k * 512:(k + 1) * 512],
                        lhsT=pkd[:],
                        rhs=rd_sb[i][:, k * 512:(k + 1) * 512],
                        start=False, stop=False)
            return state

        def emit_gh(i, kqp, att):
            den = p_sm.tile([P, 1], F32, tag="sm")
            nc.vector.tensor_scalar(out=den[:], in0=kqp[:, 0:1], scalar1=EPS,
                                    scalar2=None, op0=OP.add)
            alpha = p_sm.tile([P, 1], F32, tag="sm")
            nc.vector.reciprocal(alpha[:], den[:])
            beta = p_sm.tile([P, 1], F32, tag="sm")
            nc.vector.tensor_scalar(out=beta[:], in0=kqp[:, 0:1],
                                    scalar1=-THRESH, scalar2=THRESH,
                                    op0=OP.mult, op1=OP.add)
            o = p_o.tile([P, D], BF16, tag="o")
            nc.scalar.mul(o[:], att[:], alpha[:])
            wt = p_wt.tile([P, D], BF16, tag="wt")
            nc.gpsimd.tensor_scalar(out=wt[:], in0=wb_sb[:], scalar1=beta[:],
                                    scalar2=None, op0=OP.mult)
            o2 = p_o2.tile([P, D], BF16, tag="o2")
            nc.gpsimd.tensor_tensor(o2[:], o[:], wt[:], OP.add)
            nc.sync.dma_start(out_ap[i * P:(i + 1) * P, :], o2[:])

        abc_state = [None] * NBLK
        for step in range(NBLK + 3):
            j = step - 3
            if j >= 0:
                emit_d(j, 0)
            if step < NBLK:
                abc_state[step] = emit_a(step, (0,), None)
            if j >= 0:
                ef = emit_e(j, 0, None)
                emit_f(j, 0, ef)
                emit_d(j, 1)
            if step < NBLK:
                emit_a(step, (1,), abc_state[step])
            if j >= 0:
                emit_e(j, 1, ef)
                emit_f(j, 1, ef)
            if step < NBLK:
                emit_bc(step, *abc_state[step])
            if j >= 0:
                emit_gh(j, ef[1], ef[2])

    nc.compile()
    return nc


def get_nc():
    if "nc" not in _CACHE:
        _CACHE["nc"] = _build()
    return _CACHE["nc"]


def make_in_maps(x, W):
    import ml_dtypes
    bf = ml_dtypes.bfloat16
    f8 = ml_dtypes.float8_e4m3
    x = np.asarray(x, dtype=np.float32)
    W = np.asarray(W, dtype=np.float32)
    wrow = W.sum(axis=1, dtype=np.float32)                      # (D,)
    wb = np.ascontiguousarray(np.broadcast_to(wrow, (P, D))).astype(bf)
    idb = np.eye(P, dtype=bf)
    on8 = np.ones((P, 2, 4), dtype=f8)
    in_maps = []
    for core in range(8):
        b, h = core // 2, core % 2
        # rotate keys so this core's queries are key columns 0:QR
        xb = np.roll(x[b], -h * QR, axis=0)                     # (T, D)
        xt8 = np.ascontiguousarray(xb.T).astype(f8).reshape(D // 256, P, 2, T)
        xt0 = np.ascontiguousarray(xt8[:, :, :, :T // 2])
        xt1 = np.ascontiguousarray(xt8[:, :, :, T // 2:])
        xh8_f = xb.astype(f8)                                   # (T, D)
        # DoubleRow pairing: lhsT partition p, slot j of chunk c holds key
        # t = c*256 + j*128 + p
        xh8 = np.ascontiguousarray(
            xh8_f.reshape(NKC, 2, P, D).transpose(0, 2, 1, 3))
        rdg = (xb[:QR] - xh8_f[:QR].astype(np.float32)).astype(bf)
        rdg = np.ascontiguousarray(rdg.reshape(NBLK, P, D))
        in_maps.append({"xt0": xt0, "xt1": xt1, "xh8": xh8, "rdg": rdg,
                        "wb": wb, "idb": idb, "on8": on8})
    return in_maps


def kernel(x, W):
    nc = get_nc()
    in_maps = make_in_maps(x, W)
    res = run_bass_kernel_spmd(nc, in_maps, list(range(8)))
    out = np.empty((4, T, D), dtype=np.float32)
    for core in range(8):
        b, h = core // 2, core % 2
        out[b, h * QR:(h + 1) * QR, :] = \
            np.asarray(res.results[core]["out"]).astype(np.float32)
    return out


# revision 40
# speedup vs baseline: 2.6516x; 1.0092x over previous
"""Trainium2 Bass kernel for CasimirSparseAttention.

Math (per batch b):
    S = (x_b @ x_b.T) / sqrt(D)                      # (T, T)
    probs = softmax(S, axis=-1)
    kept = probs >= 0.01  (vacuum = probs < 0.01)
    vac_sum = sum(probs * ~kept)
    casimir[t, o] = vac_sum[t] * rowsum_W[o]          # vac_in is const across D
    attended = (probs*kept) @ x_b / (sum(probs*kept) + 1e-9)
    out = attended + 0.01 * casimir

Working in unnormalized exp-space (E = exp(S/sqrt(D)), row sum = sa):
    kept mask:  E >= 0.01 * sa
    w = (E * mask) / sa            # masked probs, cast to fp8 e4m3
    kq = sum(w8)                   # fp8-consistent normalizer (matmul ones col)
    attended = (w8 @ xh8 + w16_diag @ rdiag) / (kq + 1e-9)
    out = attended + 0.01*(1 - kq) * rowsum_W

fp8 weight quantization cancels in attended because the normalizer kq is
the sum of the SAME fp8 values the matmul contracts (ones-column matmul).
Values x are fp8 e4m3 (xh8) plus a bf16 residual correction restricted to
each q-block's diagonal key segment (rdiag = x - fp8(x), bf16 weights):
for this operator s_ii ~ sqrt(D) >> s_ij ~ O(1), so the kept mass sits on
the diagonal and the correction recovers bf16-class accuracy there at
1/4 the PE cost of a second full value stream.

Sharding: 8 cores = (batch b in 0..3) x (half of T). Keys are rotated
per-core so this core's 1024 query rows are always key columns 0:1024 —
the S lhsT is then a plain slice of the resident key tensor (no separate
query DMA). The attended values use the same rotated key order; output
rows are queries, so the rotation needs no undo.

Per-core, software-pipelined 3 blocks deep over 8 row-blocks of 128 rows.
Emission order per step interleaves PE stages of block i-3 with the S
matmuls of block i so the DVE/ACT->PE handoffs (pkb, pkt) and the input
DMA fill are latency-hidden. Stages (j = i-3), each in two groups of 4
contraction chunks:
    D(j):   PE   8 bf16 128x128 transposes of pkb into one PSUM bank
    A(i):   PE   S-column-half via fp8 DoubleRow matmuls; ACT exp->bf16
                 + row-sum accum (B: Pool thr; DVE 1/sa)
    E(j):   DVE+ACT copy transposes PSUM->SBUF casting bf16->fp8
                 (plane-contiguous => ISA-valid dual-fp8 ldweights)
    F(j):   PE   kq ones-matmul + fp8 DoubleRow att matmuls (hi)
                 + bf16 diag-residual matmul after group 0
    C(i):   DVE  mk = (E>=thr)*rsum (4x tensor_scalar), pkb = E*mk (bf16)
    G(j):   DVE  alpha = 1/(kq+1e-9), beta = 0.01*(1-kq)
    H(j):   ACT  o = att*alpha -> bf16; Pool wt = wb*beta, o2 = o+wt;
                 DMA out

The fp8 PE transpose writes element-step-2 output and dual-fp8 ldweights
require contiguous columns, so weights transpose in bf16 and the fp8 cast
rides the PSUM->SBUF copy.
"""

import sys

sys.path.insert(0, "/opt/trn_rl_repo")

from contextlib import ExitStack

import numpy as np

from concourse import bacc, mybir, tile
from concourse.bass_utils import run_bass_kernel_spmd

F32 = mybir.dt.float32
BF16 = mybir.dt.bfloat16
FP8 = mybir.dt.float8e4
U16 = mybir.dt.uint16
F16 = mybir.dt.float16
OP = mybir.AluOpType
AFT = mybir.ActivationFunctionType
DR = mybir.MatmulPerfMode.DoubleRow

P = 128          # partitions / row-block size
T = 2048         # keys per batch
D = 1024         # model dim
QR = 1024        # query rows per core
NBLK = QR // P   # 8 row blocks per core
NKC = T // 256   # 8 contraction chunks (256 wide) for attended
SCALE = float(1.0 / np.sqrt(np.float32(D)))   # 0.03125
THRESH = 0.01
EPS = 1e-9

_CACHE = {}


def _build():
    nc = bacc.Bacc("TRN2", target_bir_lowering=False, debug=False)

    # fp8 DoubleRow layout for S, split in column halves for earlier S start:
    # [chunk, K=128, 2, T/2], pairs = rows 2p, 2p+1 of x^T (rotated key order)
    xt0 = nc.dram_tensor("xt0", [D // 256, P, 2, T // 2], FP8,
                         kind="ExternalInput")
    xt1 = nc.dram_tensor("xt1", [D // 256, P, 2, T // 2], FP8,
                         kind="ExternalInput")
    # fp8 DoubleRow value layout for attended: [chunk, K=128, 2, D],
    # element [c, p, j, :] = x[c*256 + j*128 + p, :] in rotated key order
    # (hi = fp8(x), lo = fp8(x - hi))
    xh8 = nc.dram_tensor("xh8", [NKC, P, 2, D], FP8, kind="ExternalInput")
    # bf16 residuals (x - fp8(x)) for this core's own 1024 keys, one
    # [128, D] tile per q-block's diagonal segment
    rdg = nc.dram_tensor("rdg", [NBLK, P, D], BF16, kind="ExternalInput")
    wb = nc.dram_tensor("wb", [P, D], BF16, kind="ExternalInput")  # rowsum_W
    idb = nc.dram_tensor("idb", [P, P], BF16, kind="ExternalInput")
    on8 = nc.dram_tensor("on8", [P, 2, 4], FP8, kind="ExternalInput")
    out = nc.dram_tensor("out", [QR, D], BF16, kind="ExternalOutput")

    out_ap = out.ap()

    with tile.TileContext(nc) as tc, ExitStack() as ctx:
        # resident operands
        p_xt = ctx.enter_context(tc.tile_pool(name="xt", bufs=2 * (D // 256)))
        p_xv = ctx.enter_context(tc.tile_pool(name="xv", bufs=NKC))
        p_rd = ctx.enter_context(tc.tile_pool(name="rd", bufs=NBLK))
        p_pkd = ctx.enter_context(tc.tile_pool(name="pkd", bufs=2))
        p_cst = ctx.enter_context(tc.tile_pool(name="cst", bufs=3))
        # per-block working tiles
        p_ex = ctx.enter_context(tc.tile_pool(name="ex", bufs=4))
        p_mk = ctx.enter_context(tc.tile_pool(name="mk", bufs=4))
        p_pk = ctx.enter_context(tc.tile_pool(name="pk", bufs=6))
        p_pkt = ctx.enter_context(tc.tile_pool(name="pkt", bufs=2))
        p_o = ctx.enter_context(tc.tile_pool(name="o", bufs=2))
        p_o2 = ctx.enter_context(tc.tile_pool(name="o2", bufs=2))
        p_wt = ctx.enter_context(tc.tile_pool(name="wt", bufs=2))
        p_sm = ctx.enter_context(tc.tile_pool(name="sm", bufs=56))
        # PSUM: 2x(2 banks) S halves + 1 bank transposes + 2 banks att + kq
        p_ps_s = ctx.enter_context(tc.tile_pool(name="ps_s", bufs=2,
                                                space="PSUM"))
        p_ps_tp = ctx.enter_context(tc.tile_pool(name="ps_tp", bufs=1,
                                                 space="PSUM"))
        p_ps_a = ctx.enter_context(tc.tile_pool(name="ps_a", bufs=1,
                                                space="PSUM"))
        p_ps_k = ctx.enter_context(tc.tile_pool(name="ps_k", bufs=1,
                                                space="PSUM"))

        # constants + S operands first (gate the first matmul); all DMAs share
        # one 360GB/s pipe, so order = priority.
        id_sb = p_cst.tile([P, P], BF16, tag="id")
        nc.sync.dma_start(id_sb[:], idb.ap())
        on_sb = p_cst.tile([P, 2, 4], FP8, tag="on")
        nc.sync.dma_start(on_sb[:], on8.ap())
        xt_sb = [[], []]          # [half][chunk]
        for half, dram in ((0, xt0), (1, xt1)):
            for c in range(D // 256):
                t_ = p_xt.tile([P, 2, T // 2], FP8, tag="xt", name="t_xt")
                nc.sync.dma_start(t_[:], dram.ap()[c])
                xt_sb[half].append(t_)
        wb_sb = p_cst.tile([P, D], BF16, tag="wb")
        nc.sync.dma_start(wb_sb[:], wb.ap())
        # value loads: hi chunks, then per-block diag residuals
        xh_sb, rd_sb = [], []
        for c in range(NKC):
            t_ = p_xv.tile([P, 2, D], FP8, tag="xv", name="t_xh")
            nc.sync.dma_start(t_[:], xh8.ap()[c])
            xh_sb.append(t_)
        for i in range(NBLK):
            t_ = p_rd.tile([P, D], BF16, tag="rd", name="t_rd")
            nc.sync.dma_start(t_[:], rdg.ap()[i])
            rd_sb.append(t_)

        # per-block state carried across pipeline stages
        pk8_blk = [None] * NBLK
        tp_blk = [None] * NBLK

        def emit_d(i, grp):
            # 8 bf16 transposes of t-segments grp*8..grp*8+7 into one
            # 1-bank PSUM tile: plane s holds keys t = grp*1024 + s*128 + p
            # for 128 contiguous q columns.
            tp = p_ps_tp.tile([P, NKC, P], BF16, tag="tp", name="tp")
            for s in range(8):
                seg = grp * 8 + s
                src = pk8_blk[i][seg // 8][:, (seg % 8) * P:(seg % 8 + 1) * P]
                nc.tensor.transpose(tp[:, s, :], src, id_sb[:])
            tp_blk[i] = tp

        def emit_a(i, halves_sel, state):
            # S by column halves; `halves_sel` selects which to emit so
            # callers can interleave other PE work between them.
            qcols = slice(i * P, (i + 1) * P)
            if state is None:
                state = ([None, None], [])
            ex_halves, sa_parts = state
            for half in halves_sel:
                sp = p_ps_s.tile([P, T // 2], F32, tag="s", name="sp")
                for k in range(2):
                    for c in range(D // 256):
                        nc.tensor.matmul(
                            sp[:, k * 512:(k + 1) * 512],
                            lhsT=xt_sb[0][c][:, :, qcols],
                            rhs=xt_sb[half][c][:, :, k * 512:(k + 1) * 512],
                            start=(c == 0), stop=(c == D // 256 - 1),
                            perf_mode=DR)
                ex = p_ex.tile([P, T // 2], BF16, tag="ex", name="ex")
                sa = p_sm.tile([P, 1], F32, tag="sm")
                nc.scalar.activation(ex[:], sp[:], AFT.Exp, scale=SCALE,
                                     accum_out=sa[:])
                ex_halves[half] = ex
                sa_parts.append(sa)
            return state

        def emit_bc(i, ex_halves, sa_parts):
            sa = p_sm.tile([P, 1], F32, tag="sm")
            nc.gpsimd.tensor_tensor(sa[:], sa_parts[0][:], sa_parts[1][:],
                                    OP.add)
            thr = p_sm.tile([P, 1], F32, tag="sm")
            nc.gpsimd.tensor_scalar(out=thr[:], in0=sa[:], scalar1=THRESH,
                                    scalar2=None, op0=OP.mult)
            rsum = p_sm.tile([P, 1], F32, tag="sm")
            nc.vector.reciprocal(rsum[:], sa[:])
            halves = []
            for half in range(2):
                mk = p_mk.tile([P, T // 2], BF16, tag="mk")
                nc.vector.tensor_scalar(out=mk[:], in0=ex_halves[half][:],
                                        scalar1=thr[:], scalar2=rsum[:],
                                        op0=OP.is_ge, op1=OP.mult)
                pkb = p_pk.tile([P, T // 2], BF16, tag="pk")
                nc.vector.tensor_tensor(pkb[:], ex_halves[half][:], mk[:],
                                        OP.mult)
                halves.append(pkb)
            pk8_blk[i] = halves

        def emit_e(i, grp, state):
            # copy+cast bf16 -> fp8: planes [p, s, q] with contiguous q,
            # giving ISA-valid dual-fp8 DoubleRow weights [p, 2, q].
            # Split DVE/ACT so the copy latency hides under the S matmuls.
            if state is None:
                kqp = p_ps_k.tile([P, 4], F32, tag="kq", name="kqp")
                att = p_ps_a.tile([P, D], F32, tag="a", name="att")
                state = ([None, None], kqp, att, None)
            pkt = p_pkt.tile([P, NKC, P], FP8, tag="pkt", name="pkt")
            tp = tp_blk[i]
            nc.vector.tensor_copy(pkt[:, 0:4], tp[:, 0:4])
            nc.scalar.copy(pkt[:, 4:8], tp[:, 4:8])
            if grp == 0:
                # diag segment of block i is plane i of group 0: keep the
                # bf16 weights for the residual correction matmul
                pkd = p_pkd.tile([P, P], BF16, tag="pkd", name="pkd")
                nc.vector.tensor_copy(pkd[:], tp[:, i, :])
                state = (state[0], state[1], state[2], pkd)
            state[0][grp] = pkt
            return state

        def emit_f(i, grp, state):
            # kq + att matmuls for this group's 4 contraction chunks,
            # plus the bf16 diagonal-residual correction after group 0.
            pkts, kqp, att, pkd = state
            pkt = pkts[grp]
            for cl in range(NKC // 2):
                c = grp * (NKC // 2) + cl
                lhs = pkt[:, 2 * cl:2 * cl + 2, :]
                nc.tensor.matmul(kqp[:], lhsT=lhs,
                                 rhs=on_sb[:], start=(c == 0),
                                 stop=(c == NKC - 1), perf_mode=DR)
                for k in range(2):
                    nc.tensor.matmul(
                        att[:, k * 512:(k + 1) * 512],
                        lhsT=lhs,
                        rhs=xh_sb[c][:, :, k * 512:(k + 1) * 512],
                        start=(c == 0), stop=(c == NKC - 1),
                        perf_mode=DR)
            if grp == 0:
                for k in range(2):
                    nc.tensor.matmul(
                        att[:, k * 512:(k + 1) * 512],
                        lhsT=pkd[:],
                        rhs=rd_sb[i][:, k * 512:(k + 1) * 512],
                        start=False, stop=False)
            return state

        def emit_gh(i, kqp, att):
            den = p_sm.tile([P, 1], F32, tag="sm")
            nc.vector.tensor_scalar(out=den[:], in0=kqp[:, 0:1], scalar1=EPS,
                                    scalar2=None, op0=OP.add)
            alpha = p_sm.tile([P, 1], F32, tag="sm")
            nc.vector.reciprocal(alpha[:], den[:])
            beta = p_sm.tile([P, 1], F32, tag="sm")
            nc.vector.tensor_scalar(out=beta[:], in0=kqp[:, 0:1],
                                    scalar1=-THRESH, scalar2=THRESH,
                                    op0=OP.mult, op1=OP.add)
            o = p_o.tile([P, D], BF16, tag="o")
            nc.scalar.mul(o[:, 0:512], att[:, 0:512], alpha[:])
            nc.vector.tensor_scalar(out=o[:, 512:D], in0=att[:, 512:D],
                                    scalar1=alpha[:], scalar2=None,
                                    op0=OP.mult)
            wt = p_wt.tile([P, D], BF16, tag="wt")
            nc.gpsimd.tensor_scalar(out=wt[:], in0=wb_sb[:], scalar1=beta[:],
                                    scalar2=None, op0=OP.mult)
            o2 = p_o2.tile([P, D], BF16, tag="o2")
            nc.gpsimd.tensor_tensor(o2[:], o[:], wt[:], OP.add)
            nc.sync.dma_start(out_ap[i * P:(i + 1) * P, :], o2[:])

        abc_state = [None] * NBLK
        for step in range(NBLK + 3):
            j = step - 3
            if j >= 0:
                emit_d(j, 0)
            if step < NBLK:
                abc_state[step] = emit_a(step, (0,), None)
            if j >= 0:
                ef = emit_e(j, 0, None)
                emit_f(j, 0, ef)
                emit_d(j, 1)
            if step < NBLK:
                emit_a(step, (1,), abc_state[step])
            if j >= 0:
                emit_e(j, 1, ef)
                emit_f(j, 1, ef)
            if step < NBLK:
                emit_bc(step, *abc_state[step])
            if j >= 0:
                emit_gh(j, ef[1], ef[2])

    nc.compile()
    return nc


def get_nc():
    if "nc" not in _CACHE:
        _CACHE["nc"] = _build()
    return _CACHE["nc"]


def make_in_maps(x, W):
    import ml_dtypes
    bf = ml_dtypes.bfloat16
    f8 = ml_dtypes.float8_e4m3
    x = np.asarray(x, dtype=np.float32)
    W = np.asarray(W, dtype=np.float32)
    wrow = W.sum(axis=1, dtype=np.float32)                      # (D,)
    wb = np.ascontiguousarray(np.broadcast_to(wrow, (P, D))).astype(bf)
    idb = np.eye(P, dtype=bf)
    on8 = np.ones((P, 2, 4), dtype=f8)
    in_maps = []
    for core in range(8):
        b, h = core // 2, core % 2
        # rotate keys so this core's queries are key columns 0:QR
        xb = np.roll(x[b], -h * QR, axis=0)                     # (T, D)
        xt8 = np.ascontiguousarray(xb.T).astype(f8).reshape(D // 256, P, 2, T)
        xt0 = np.ascontiguousarray(xt8[:, :, :, :T // 2])
        xt1 = np.ascontiguousarray(xt8[:, :, :, T // 2:])
        xh8_f = xb.astype(f8)                                   # (T, D)
        # DoubleRow pairing: lhsT partition p, slot j of chunk c holds key
        # t = c*256 + j*128 + p
        xh8 = np.ascontiguousarray(
            xh8_f.reshape(NKC, 2, P, D).transpose(0, 2, 1, 3))
        rdg = (xb[:QR] - xh8_f[:QR].astype(np.float32)).astype(bf)
        rdg = np.ascontiguousarray(rdg.reshape(NBLK, P, D))
        in_maps.append({"xt0": xt0, "xt1": xt1, "xh8": xh8, "rdg": rdg,
                        "wb": wb, "idb": idb, "on8": on8})
    return in_maps


def kernel(x, W):
    nc = get_nc()
    in_maps = make_in_maps(x, W)
    res = run_bass_kernel_spmd(nc, in_maps, list(range(8)))
    out = np.empty((4, T, D), dtype=np.float32)
    for core in range(8):
        b, h = core // 2, core % 2
        out[b, h * QR:(h + 1) * QR, :] = \
            np.asarray(res.results[core]["out"]).astype(np.float32)
    return out
